# revision 21
# baseline (speedup 1.0000x reference)
"""Grouped per-sample MLP (conv1d groups=B) + GroupSwish + softmax, on 8 NeuronCores.

Data-parallel over the group/batch axis B=256: 32 groups per core.
Per group g: h = W1[g] @ x[g] + b1[g]; GroupSwish; o = W2[g] @ h + b2[g];
softmax over the flattened [C*L] logits.

Device strategy per core (per group, fully unrolled):
  - W1 matmul out[32, 512], contraction X=784 split 6x128 + 16, operands fed
    as float32r (TF32-like, 1 PE cycle/row, HW rounds internally) straight
    from DMA. fp32r matmuls must write PSUM at partition base 0, so groups
    are not column-packed.
  - x rows are distributed so each partition's x load is one contiguous DRAM
    run (partitions 0..16 carry 7 rows = 14KB, partitions 16..128 carry 6
    rows = 12KB; K-chunk 6 only spans partitions 0..16). The row permutation
    is absorbed into the host-marshalled W1T layout.
  - GroupSwish via tanh (the only ACT table with both tanh and exp):
    (h+b1)*sigmoid(sp*(h+b1)) = ((h+b1)*0.5) * (1 + tanh(sp*(h+b1)/2)).
    The 1/1.1 factor is folded into W2 host-side; sp = softplus(beta) is
    computed on device via exp/ln.
  - Softmax without max-subtraction (logits are O(1)): exp with fused
    per-partition accum, cross-partition sum / broadcast via tiny matmuls
    against ones vectors.
"""

import os
import numpy as np
from contextlib import ExitStack

import concourse.mybir as mybir
import concourse.tile as tile
from concourse import bacc
from concourse.bass_utils import run_bass_kernel_spmd

B, X, Z, C, L = 256, 784, 32, 10, 512
NCORE = 8
GPC = B // NCORE  # 32 groups per core
NCH = 7  # K-chunks: 6*128 + 16
KLAST = X - 6 * 128  # 16
P = 128
F32 = mybir.dt.float32
F32R = mybir.dt.float32r

# DVE-produced swish as float32r (full-rate W2 matmul); flip False to use a
# plain fp32 W2 matmul if the fp32r producer constraint rejects DVE output.
SWISH_F32R = True

_CACHE: dict = {}


def _build():
    nc = bacc.Bacc("TRN2", target_bir_lowering=False, debug=False)

    xg = nc.dram_tensor("xg", [GPC, X, L], F32R, kind="ExternalInput").ap()
    # W1T: [group-quad, partition, group-in-quad, chunk, z] so each partition
    # reads one contiguous 4*7*32*4B run per quad-DMA (chunk 6 rows beyond
    # partition 16 are zero padding, never read by the matmuls)
    w1m = nc.dram_tensor(
        "w1m", [GPC // 4, P, 4, NCH, Z], F32R, kind="ExternalInput"
    ).ap()
    w2t = nc.dram_tensor(
        "w2t", [Z, GPC * C], F32R if SWISH_F32R else F32, kind="ExternalInput"
    ).ap()
    b1c = nc.dram_tensor("b1c", [Z, GPC], F32, kind="ExternalInput").ap()
    btc = nc.dram_tensor("btc", [Z, GPC], F32, kind="ExternalInput").ap()
    b2c = nc.dram_tensor("b2c", [C, GPC], F32, kind="ExternalInput").ap()
    out = nc.dram_tensor("out", [GPC, C, L], F32, kind="ExternalOutput").ap()

    with tile.TileContext(nc) as tc, ExitStack() as ctx:
        consts = ctx.enter_context(tc.tile_pool(name="consts", bufs=1))
        xpool = ctx.enter_context(tc.tile_pool(name="x", bufs=6))
        wpool = ctx.enter_context(tc.tile_pool(name="w1", bufs=3))
        spool = ctx.enter_context(tc.tile_pool(name="act", bufs=3))
        hps = ctx.enter_context(tc.tile_pool(name="hps", bufs=3, space="PSUM"))
        ops = ctx.enter_context(tc.tile_pool(name="ops", bufs=2, space="PSUM"))
        tps = ctx.enter_context(tc.tile_pool(name="tps", bufs=2, space="PSUM"))

        # --- constants / per-group scalars ---
        w2tt = consts.tile([Z, GPC * C], F32R if SWISH_F32R else F32, name="w2tt")
        nc.gpsimd.dma_start(w2tt[:], w2t)
        b1t = consts.tile([Z, GPC], F32, name="b1t")
        nc.gpsimd.dma_start(b1t[:], b1c)
        btt = consts.tile([Z, GPC], F32, name="btt")
        nc.gpsimd.dma_start(btt[:], btc)
        b2t = consts.tile([C, GPC], F32, name="b2t")
        nc.gpsimd.dma_start(b2t[:], b2c)
        ones_k = consts.tile([C, 1], F32, name="ones_k")
        nc.vector.memset(ones_k[:], 1.0)
        ones_m = consts.tile([1, C], F32, name="ones_m")
        nc.vector.memset(ones_m[:], 1.0)

        # sp = softplus(beta) = ln(1 + exp(beta)); halves for tanh-sigmoid
        spe = consts.tile([Z, GPC], F32, name="spe")
        nc.scalar.activation(spe[:], btt[:], mybir.ActivationFunctionType.Exp)
        spe1 = consts.tile([Z, GPC], F32, name="spe1")
        nc.vector.tensor_scalar_add(spe1[:], spe[:], 1.0)
        spt = consts.tile([Z, GPC], F32, name="spt")
        nc.scalar.activation(spt[:], spe1[:], mybir.ActivationFunctionType.Ln)
        sph = consts.tile([Z, GPC], F32, name="sph")
        nc.vector.tensor_scalar_mul(sph[:], spt[:], 0.5)
        spb1h = consts.tile([Z, GPC], F32, name="spb1h")
        nc.vector.tensor_mul(spb1h[:], sph[:], b1t[:])

        for g in range(GPC):
            gq, jq = divmod(g, 4)
            # --- load x for group g as one contiguous run per partition:
            # partitions 0..16 hold rows 7p..7p+7, partitions 16..128 hold
            # rows 112+6(p-16)..+6
            xt = xpool.tile([P, NCH * L], F32R, tag="xt", name=f"xt{g}")
            nc.sync.dma_start(
                xt[:16, :].rearrange("p (c l) -> p c l", c=NCH),
                xg[g, : 7 * 16].rearrange("(p c) l -> p c l", p=16),
            )
            nc.sync.dma_start(
                xt[16:, : 6 * L].rearrange("p (c l) -> p c l", c=6),
                xg[g, 7 * 16 : X].rearrange("(p c) l -> p c l", p=112),
            )
            # --- W1T for a quad of 4 groups, one DMA every 4th group ---
            if jq == 0:
                wt = wpool.tile([P, 4 * NCH * Z], F32R, tag="wt", name=f"wt{g}")
                nc.gpsimd.dma_start(
                    wt[:].rearrange("p (j c z) -> p j c z", j=4, c=NCH),
                    w1m[gq],
                )

            # --- h = W1 @ x ---
            h = hps.tile([Z, L], F32, tag="h", name=f"h{g}")
            for c in range(NCH):
                kk = P if c < 6 else KLAST
                nc.tensor.matmul(
                    h[:],
                    wt[:kk, (jq * NCH + c) * Z : (jq * NCH + c + 1) * Z],
                    xt[:kk, c * L : (c + 1) * L],
                    start=(c == 0),
                    stop=(c == NCH - 1),
                )

            # --- GroupSwish: ((h+b1)*0.5) * (1 + tanh(sp*(h+b1)/2)) ---
            t = spool.tile([Z, L], F32, tag="t", name=f"t{g}")
            nc.scalar.activation(
                t[:],
                h[:],
                mybir.ActivationFunctionType.Tanh,
                bias=spb1h[:, g : g + 1],
                scale=sph[:, g : g + 1],
            )
            u = spool.tile([Z, L], F32, tag="u", name=f"u{g}")
            nc.vector.tensor_scalar(
                u[:],
                h[:],
                b1t[:, g : g + 1],
                0.5,
                op0=mybir.AluOpType.add,
                op1=mybir.AluOpType.mult,
            )
            swish = spool.tile(
                [Z, L], F32R if SWISH_F32R else F32, tag="swish", name=f"sw{g}"
            )
            nc.vector.scalar_tensor_tensor(
                swish[:],
                t[:],
                1.0,
                u[:],
                op0=mybir.AluOpType.add,
                op1=mybir.AluOpType.mult,
            )

            # --- o = (W2/1.1) @ swish ---
            o = ops.tile([C, L], F32, tag="o", name=f"o{g}")
            nc.tensor.matmul(
                o[:],
                w2tt[:, g * C : (g + 1) * C],
                swish[:],
                start=True,
                stop=True,
            )

            # --- softmax over [C, L] (no max subtraction) ---
            expo = spool.tile([C, L], F32, tag="expo", name=f"e{g}")
            esum = spool.tile([C, 1], F32, tag="esum", name=f"es{g}")
            nc.scalar.activation(
                expo[:],
                o[:],
                mybir.ActivationFunctionType.Exp,
                bias=b2t[:, g : g + 1],
                scale=1.0,
                accum_out=esum[:],
            )
            tot = tps.tile([1, 1], F32, tag="tb", name=f"tot{g}")
            nc.tensor.matmul(tot[:], ones_k[:], esum[:], start=True, stop=True)
            inv = spool.tile([1, 1], F32, tag="inv", name=f"inv{g}")
            nc.vector.reciprocal(inv[:], tot[:])
            bc = tps.tile([C, 1], F32, tag="tb", name=f"bc{g}")
            nc.tensor.matmul(bc[:], ones_m[:], inv[:], start=True, stop=True)
            invc = spool.tile([C, 1], F32, tag="invc", name=f"ic{g}")
            nc.vector.tensor_copy(invc[:], bc[:])
            res = spool.tile([C, L], F32, tag="res", name=f"r{g}")
            nc.vector.tensor_scalar_mul(res[:], expo[:], invc[:])

            nc.gpsimd.dma_start(out[g], res[:])

    nc.compile()
    return nc


def _marshal(x, W1, b1, beta, W2, b2):
    """Full inputs -> list of per-core input dicts."""
    xg = np.ascontiguousarray(x, dtype=np.float32).reshape(B, X, L)
    w1T = W1.astype(np.float32, copy=False).transpose(0, 2, 1)  # [B, X, Z]
    # w1m[gq, p, j, c, z] = W1T[4gq+j, row(p, c), z];
    # row(p, c) = 7p+c for p<16 (7 chunks), 112+6(p-16)+c for p>=16 (6 chunks)
    w1m = np.zeros((B // 4, P, 4, NCH, Z), np.float32)
    lo = w1T[:, : 7 * 16].reshape(B // 4, 4, 16, NCH, Z)  # [gq, j, p, c, z]
    hi = w1T[:, 7 * 16 :].reshape(B // 4, 4, 112, 6, Z)
    w1m[:, :16] = lo.transpose(0, 2, 1, 3, 4)
    w1m[:, 16:, :, :6] = hi.transpose(0, 2, 1, 3, 4)
    w2s = (W2.astype(np.float32, copy=False) * np.float32(1.0 / 1.1)).transpose(
        0, 2, 1
    )  # [B, Z, C]

    in_maps = []
    for core in range(NCORE):
        s = slice(core * GPC, (core + 1) * GPC)
        sq = slice(core * GPC // 4, (core + 1) * GPC // 4)
        in_maps.append(
            {
                "xg": xg[s],
                "w1m": w1m[sq],
                # [Z, GPC*C]: w2t[z, g*C+c] = W2[g0+g, c, z] / 1.1
                "w2t": np.ascontiguousarray(
                    w2s[s].transpose(1, 0, 2).reshape(Z, GPC * C)
                ),
                "b1c": np.ascontiguousarray(b1[s].astype(np.float32).T),  # [Z, GPC]
                "btc": np.ascontiguousarray(
                    np.broadcast_to(beta[s].astype(np.float32), (Z, GPC))
                ),
                "b2c": np.ascontiguousarray(b2[s].astype(np.float32).T),  # [C, GPC]
            }
        )
    return in_maps


def _run(in_maps, trace=False, tmpdir=None):
    if "nc" not in _CACHE:
        _CACHE["nc"] = _build()
    return run_bass_kernel_spmd(
        _CACHE["nc"],
        in_maps,
        core_ids=list(range(NCORE)),
        trace=trace,
        tmpdir=tmpdir,
    )


_LAST = {}


def kernel(x, W1, b1, beta, W2, b2):
    in_maps = _marshal(x, W1, b1, beta, W2, b2)
    trace = bool(os.environ.get("KERNEL_TRACE"))
    r = _run(in_maps, trace=trace, tmpdir=os.environ.get("KERNEL_TRACE_DIR"))
    _LAST["results"] = r
    outs = [r.results[c]["out"].reshape(GPC, C * L) for c in range(NCORE)]
    return np.concatenate(outs, axis=0)
